# revision 39
# baseline (speedup 1.0000x reference)
"""Trainium2 Bass kernel for nn_DecGreenNet_product_CP3.

Reference computation:
    lhs  = tanh(input @ Wx1 + bx1) @ Wx2 + bx2          # [B, 512]
    s_i  = sum_n sin(pi*eq*qx_n) * mlp_i(qx_n)           # [8,16] per branch
    rhs  = einsum('bx,dx,fx->bdf', s_a, s_c, s_e)        # [512]
    out  = lhs @ rhs                                     # [B]

Algebraic restructuring (validated to ~3e-3 rel err):
    out[b] = tanh(input[b] @ Wx1 + bx1) @ (Wx2 @ rhs) + bx2 @ rhs
    s      = W2^T @ (h1tanh^T @ y) + (sum y) * b2   per quad branch
collapsing the dominant [B,512]x[512,512] GEMM into a matvec.

Implementation notes (v8):
  * Batch B sharded 8 ways (8192 rows/core). The quadrature branch is
    small (3 x 8192 x 128 tanh = 75% of one batch-shard's tanh work),
    so every core computes the FULL quadrature instead of exchanging
    partial sums: measured launch skew between the 8 cores is 40-60us
    per run, which made any cross-core reduction (AllReduce/AllGather/
    p2p remote_dma) the critical path. Fully local compute is ~20us
    slower in ScalarE terms but deterministic and variance-free.
  * The L1 GEMM has contraction K=4 (3 dims + bias) and runs as 4
    concurrent 32x128 row-tiles of the PE array (tile_position), with x
    and W replicated at partition offsets 0/32/64/96. The quadrature
    first layer (K=2) uses 2 row-tiles.
  * tanh runs in [128,1024] tiles from 3 rotating 2-bank PSUM slots;
    ScalarE is the roofline engine (56 tiles ~ 64us).
  * z = h1^T y runs with y as the stationary operand (1-column weight
    load) producing z as rows; a small SBUF gather DMA transposes the
    three z rows into matmul-rhs columns.
  * s lands directly in the [16, 24] einsum layout via M=16 matmuls on
    4 PE column-tiles. w = Wx2 @ rhs uses K=64 blocks on 2 row-tiles.
  * The final dot uses w replicated to 32 columns; each chunk lands as
    a 32-row PSUM stripe and 4 chunks fill a [128,512] bank evacuated
    with one DVE copy. Explicit scheduler deps pin the tensor-queue
    order quad -> L1 -> dots, with the s/einsum/w block slotted into
    the middle of the L1 stream (its inputs are ready long before).
  * The scalar c = bx2 @ rhs and the 2^36 fp16 descale are applied on
    the host; the kernel dumps the fp32 rhs vector in a 512-float tail
    of the output buffer.
"""

import os

import numpy as np

import concourse.bacc as bacc
import concourse.bass as bass
import concourse.mybir as mybir
import concourse.tile as tile
from concourse.bass_utils import run_bass_kernel_spmd
from concourse.tile import add_dep_helper

F32 = mybir.dt.float32
F16 = mybir.dt.float16
AF = mybir.ActivationFunctionType
ALU = mybir.AluOpType

NCORES = 8
B, DIN, H = 65536, 3, 512
N, HQ = 8192, 128
S0, RX = 8, 16
BL = B // NCORES          # 8192 batch rows per core
CH = 512                  # batch chunk (columns per L1 matmul)
NCH = BL // CH            # 16 chunks
NU = 2 * NCH              # 32 L1 units of [128, 1024]
NQU = 24                  # 24 quad units of [128, 1024] (3 br x 8 segs)

# fp16 scaling: w values are ~1e10-1e11; scale into fp16 range (exact pow2)
RC_SCALE = 2.0 ** -36     # applied to rhs_vec before the fp16 w-matmuls
OUT_SCALE = 2.0 ** 36     # undone on the host

# minimax odd polynomial for sin(t), t in [0, pi]: sin(t)=t*P(t^2), err<2e-5
SIN_C = (0.999984590176674, -0.16663258473611252, 8.312385898666645e-03,
         -1.9316230946716391e-04, 2.1732361127812407e-06)

_CACHED_NC = None
_STAGE = os.environ.get("K_STAGE", "full")  # sT | full


def _build():
    nc = bacc.Bacc("TRN2", target_bir_lowering=False, debug=False,
                   num_devices=NCORES)

    xT = nc.dram_tensor("xT", [DIN + 1, BL], F16, kind="ExternalInput").ap()
    wx1d = nc.dram_tensor("wx1d", [16, 128], F16, kind="ExternalInput").ap()
    wx2q = nc.dram_tensor("wx2q", [128, 2048], F16, kind="ExternalInput").ap()
    qxad = nc.dram_tensor("qxad", [4, 12288], F16, kind="ExternalInput").ap()
    wqad = nc.dram_tensor("wqad", [2, 384], F16, kind="ExternalInput").ap()
    qxc = nc.dram_tensor("qxc", [128, 192], F32, kind="ExternalInput").ap()
    wq2 = nc.dram_tensor("wq2", [HQ, 3 * HQ], F16, kind="ExternalInput").ap()
    bq2r = nc.dram_tensor("bq2r", [HQ, 3 * HQ], F16,
                          kind="ExternalInput").ap()
    eqb = nc.dram_tensor("eqb", [128, 1], F32, kind="ExternalInput").ap()
    out_d = nc.dram_tensor("out", [BL + 512], F32, kind="ExternalOutput").ap()

    global _APS
    _APS = (xT, wx1d, wx2q, qxad, wqad, qxc, wq2, bq2r, eqb, out_d)
    with tile.TileContext(nc) as tc:
        _body(nc, tc)
    nc.compile()
    return nc


def _body(nc, tc):
    xT, wx1d, wx2q, qxad, wqad, qxc, wq2, bq2r, eqb, out_d = _APS
    with (
        tc.tile_pool(name="const", bufs=1) as constp,
        tc.tile_pool(name="qsb", bufs=1) as qsb,
        tc.tile_pool(name="mainsb", bufs=1) as mainsb,
        tc.tile_pool(name="h1p", bufs=3) as h1p,
        tc.tile_pool(name="hidp", bufs=NU) as hidp,
        tc.tile_pool(name="oevp", bufs=2) as oevp,
        tc.tile_pool(name="prep", bufs=3, space="PSUM") as prep,
        tc.tile_pool(name="tinyp", bufs=1, space="PSUM") as tinyp,
        tc.tile_pool(name="outp", bufs=1, space="PSUM") as outp,
    ):
        ones128 = constp.tile([128, 1], F32)
        nc.vector.memset(ones128, 1.0)

        # ---------------- input DMAs ----------------
        # y-polynomial + xT on the scalar queue (idle until the first tanh)
        qxc_sb = qsb.tile([128, 192], F32, tag="qxc")
        nc.scalar.dma_start(out=qxc_sb, in_=qxc)
        eqb_sb = qsb.tile([128, 1], F32, tag="eqb")
        nc.scalar.dma_start(out=eqb_sb, in_=eqb)
        xTq = mainsb.tile([128, BL], F16, tag="xTq")
        for i in range(4):
            nc.scalar.dma_start(out=xTq[32 * i:32 * i + 4, :], in_=xT)

        # quad matmul inputs first on sync (they gate the first tanh);
        # qxaq arrives branch-by-branch so unit 0 isn't gated on all 96KB
        wqaq = qsb.tile([128, 384], F16, tag="wqaq")
        for s in range(2):
            nc.sync.dma_start(out=wqaq[32 * s:32 * s + 2, :], in_=wqad)
        qxaq = qsb.tile([128, 12288], F16, tag="qxaq")
        for br in range(3):
            for s in range(2):
                nc.sync.dma_start(
                    out=qxaq[32 * s:32 * s + 2, br * 4096:(br + 1) * 4096],
                    in_=qxad[2 * s:2 * s + 2, br * 4096:(br + 1) * 4096])
        wx1b = mainsb.tile([128, 128], F16, tag="wx1b")
        for i in range(4):
            nc.sync.dma_start(out=wx1b[32 * i:32 * i + 4, :],
                              in_=wx1d[4 * i:4 * i + 4, :])
        wq2_sb = qsb.tile([HQ, 3 * HQ], F16, tag="wq2")
        nc.sync.dma_start(out=wq2_sb, in_=wq2)
        bq2r_sb = qsb.tile([HQ, 3 * HQ], F16, tag="bq2r")
        nc.sync.dma_start(out=bq2r_sb, in_=bq2r)
        wx2q_sb = mainsb.tile([128, 2048], F16, tag="wx2q")
        nc.sync.dma_start(out=wx2q_sb, in_=wx2q)

        # ---------------- y = sin(pi*eq*qx) (DVE polynomial) ----------------
        eqpi = qsb.tile([128, 1], F32, tag="eqpi")
        nc.vector.tensor_scalar_mul(eqpi, eqb_sb, float(np.pi))
        tq = qsb.tile([128, 192], F32, tag="tq")
        nc.vector.tensor_scalar_mul(tq, qxc_sb, eqpi[:, 0:1])
        t2 = qsb.tile([128, 192], F32, tag="t2")
        nc.vector.tensor_tensor(out=t2, in0=tq, in1=tq, op=ALU.mult)
        pp = qsb.tile([128, 192], F32, tag="pp")
        c1, c3, c5, c7, c9 = [float(v) for v in SIN_C]
        nc.vector.tensor_scalar(out=pp, in0=t2, scalar1=c9, scalar2=c7,
                                op0=ALU.mult, op1=ALU.add)
        for cof in (c5, c3, c1):
            nc.vector.tensor_tensor(out=pp, in0=pp, in1=t2, op=ALU.mult)
            nc.vector.tensor_scalar_add(pp, pp, cof)
        y_sb = qsb.tile([128, 192], F16, tag="ysb")
        nc.vector.tensor_tensor(out=y_sb, in0=pp, in1=tq, op=ALU.mult)

        # ---------------- quad units ----------------
        # unit u = (br, seg): 8 node-tiles of 128 nodes; local tile k at
        # qpre cols (k%2)*512 + (k//2)*128, global node-tile g = seg*8+k.
        # zrow[0, br*128+h] accumulates z over all 64 node-tiles; cols
        # 384+br hold sum(y).
        # zsmall cols 0-23: per-unit z partials; cols 24-26: sum(y) (row 0)
        zsmall = tinyp.tile([128, 27], F32, tag="tiny")
        last_quad_mm = None
        # sum(y) scalars first: they are ready as soon as y is, and the
        # L1 stream dependency should not wait for them at the end
        for br in range(3):
            ysum = qsb.tile([128, 1], F32, tag=f"ysum{br}")
            nc.vector.tensor_reduce(
                out=ysum, in_=y_sb[:, br * 64:(br + 1) * 64],
                axis=mybir.AxisListType.X, op=ALU.add)
            nc.tensor.matmul(
                zsmall[0:1, 24 + br:25 + br], lhsT=ysum, rhs=ones128,
                start=True, stop=True, tile_position=(0, 0))
        for u in range(NQU):
            br, seg = u // 8, u % 8
            qpre = prep.tile([128, 1024], F32, tag="pre")
            for k in range(8):
                s, r = k % 2, k // 2
                col = (br * 8 + seg) * 512 + r * 128
                nc.tensor.matmul(
                    qpre[:, s * 512 + r * 128:s * 512 + (r + 1) * 128],
                    lhsT=qxaq[32 * s:32 * s + 2, col:col + 128],
                    rhs=wqaq[32 * s:32 * s + 2, br * HQ:(br + 1) * HQ],
                    start=True, stop=True, tile_position=(32 * s, 0))
            h1 = h1p.tile([128, 1024], F16, tag="h1")
            nc.scalar.activation(out=h1, in_=qpre, func=AF.Tanh)
            # z partial for this unit: z[h] = sum_n h1[n, h] y[n]
            for k in range(8):
                s, r = k % 2, k // 2
                g = seg * 8 + k
                last_quad_mm = nc.tensor.matmul(
                    zsmall[:, u:u + 1],
                    lhsT=h1[:, s * 512 + r * 128:s * 512 + (r + 1) * 128],
                    rhs=y_sb[:, br * 64 + g:br * 64 + g + 1],
                    start=(k == 0), stop=(k == 7), tile_position=(0, 0))
        # fold the 8 segment partials per branch; sy scalars to row 0
        zq32 = qsb.tile([128, 6], F32, tag="zq32")
        nc.vector.memset(zq32, 0.0)
        for br in range(3):
            nc.vector.tensor_reduce(
                out=zq32[:, br:br + 1], in_=zsmall[:, br * 8:(br + 1) * 8],
                axis=mybir.AxisListType.X, op=ALU.add)
        nc.vector.tensor_copy(out=zq32[0:1, 3:6], in_=zsmall[0:1, 24:27])
        zcol = qsb.tile([128, 6], F16, tag="zcol")
        nc.vector.tensor_copy(out=zcol, in_=zq32)
        if _STAGE == "z":
            nc.sync.dma_start(
                out=out_d[0:768].rearrange("(p c) -> p c", c=6), in_=zq32)
            return

        # ---------------- main L1 units ----------------
        hid_tiles = {}
        l1_mms = {}

        def emit_l1(u):
            c, half = u // 2, u % 2
            pre = prep.tile([128, 1024], F32, tag="pre")
            for k in range(2):
                i = 2 * half + k          # PE row strip / h-tile index
                mm = nc.tensor.matmul(
                    pre[:, k * 512:(k + 1) * 512],
                    lhsT=wx1b[32 * i:32 * i + 4, :],
                    rhs=xTq[32 * i:32 * i + 4, c * CH:(c + 1) * CH],
                    start=True, stop=True, tile_position=(32 * i, 0))
                if u == 0 and k == 0:
                    add_dep_helper(mm.ins, last_quad_mm.ins, sync=False,
                                   reason="L1 stream after quad matmuls")
                l1_mms[u] = mm
            hid = hidp.tile([128, 1024], F16, tag="hid")
            nc.scalar.activation(out=hid, in_=pre, func=AF.Tanh)
            hid_tiles[u] = hid

        for u in range(0, 8):
            emit_l1(u)

        # ---------------- s / einsum / w (all local) ----------------
        # slotted here in the tensor queue: its inputs are ready right
        # after the quad units, and the dots only need w much later.
        # s[b*16+x, br] lands in the [16, 24] einsum layout: column pair
        # cb = br*8+b -> PE col strip j=cb//6, col c=cb%6.
        sp16 = tinyp.tile([128, 6], F32, tag="tiny")
        first_s = None
        for cb in range(24):
            br, b = cb // 8, cb % 8
            j, c = cb // 6, cb % 6
            mm = nc.tensor.matmul(
                sp16[32 * j:32 * j + 16, c:c + 1],
                lhsT=wq2_sb[:, br * HQ + b * 16:br * HQ + (b + 1) * 16],
                rhs=zcol[:, br:br + 1], start=True, stop=False,
                tile_position=(0, 32 * j))
            if first_s is None:
                first_s = mm
                add_dep_helper(mm.ins, l1_mms[7].ins, sync=False,
                               reason="s block mid-L1-stream")
            nc.tensor.matmul(
                sp16[32 * j:32 * j + 16, c:c + 1],
                lhsT=bq2r_sb[:, br * HQ + b * 16:br * HQ + (b + 1) * 16],
                rhs=zcol[:, 3 + br:4 + br], start=False, stop=True,
                tile_position=(0, 32 * j))
        sSB = qsb.tile([128, 6], F32, tag="sSB")
        nc.vector.tensor_copy(out=sSB, in_=sp16)
        sT = mainsb.tile([16, 24], F32, tag="sT")
        for j in range(4):
            nc.sync.dma_start(out=sT[0:16, 6 * j:6 * (j + 1)],
                              in_=sSB[32 * j:32 * j + 16, :])
        if _STAGE == "sT":
            nc.sync.dma_start(
                out=out_d[0:384].rearrange("(p c) -> p c", c=24), in_=sT)
            return
        # E[x, d*8+f] = s_c[d,x] * s_e[f,x]
        sc_ap = sT[:, 8:16]
        se_ap = sT[:, 16:24]
        in0 = bass.AP(tensor=sc_ap.tensor, offset=sc_ap.offset,
                      ap=[sc_ap.ap[0], sc_ap.ap[1], [0, 8]])
        in1 = bass.AP(tensor=se_ap.tensor, offset=se_ap.offset,
                      ap=[se_ap.ap[0], [0, 8], se_ap.ap[1]])
        E_sb = mainsb.tile([16, 64], F32, tag="E")
        nc.vector.tensor_tensor(
            out=E_sb.rearrange("p (d f) -> p d f", f=8),
            in0=in0, in1=in1, op=ALU.mult)
        # rhsp[d*8+f, b] = sum_x E[x, df] * s_a[b, x]
        rhsp = tinyp.tile([64, 8], F32, tag="tiny")
        nc.tensor.matmul(rhsp, lhsT=E_sb, rhs=sT[:, 0:8],
                         start=True, stop=True, tile_position=(0, 0))
        r16 = mainsb.tile([64, 8], F16, tag="r16")
        nc.vector.tensor_scalar_mul(r16, rhsp, float(RC_SCALE))
        # fp32 rhs vector to the output tail (host computes bx2 @ rhs)
        rf32 = mainsb.tile([64, 8], F32, tag="rf32")
        nc.vector.tensor_copy(out=rf32, in_=rhsp)
        nc.sync.dma_start(
            out=out_d[BL:BL + 512].rearrange("(p c) -> p c", c=8),
            in_=rf32)
        # replicate r16 to partitions 64-127 for the 2-strip w matmuls:
        # strip s (partitions 64s+df) holds cols q -> b = 2q+s
        rcol2 = mainsb.tile([128, 4], F16, tag="rcol2")
        r16v = r16.rearrange("p (q b2) -> p b2 q", b2=2)
        nc.sync.dma_start(out=rcol2[0:64, :], in_=r16v[:, 0:1, :])
        nc.sync.dma_start(out=rcol2[64:128, :], in_=r16v[:, 1:2, :])
        # w[t*128+h] = sum_b sum_df Wx2[t*128+h, b*64+df] rhs[b*64+df]
        wps0 = tinyp.tile([128, 4], F32, tag="tiny")
        wps8 = outp.tile([128, 4], F32, tag="wps8")
        wtile = [wps0, wps8]
        for t in range(4):
            for bp in range(4):
                for s in range(2):
                    nc.tensor.matmul(
                        wtile[s][:, t:t + 1],
                        lhsT=wx2q_sb[64 * s:64 * s + 64,
                                     (bp * 4 + t) * 128:(bp * 4 + t + 1) * 128],
                        rhs=rcol2[64 * s:64 * s + 64, bp:bp + 1],
                        start=(bp == 0), stop=(bp == 3),
                        tile_position=(64 * s, 0))
        # w16 = wps0 + wps8, then replicate each column 32x for the dots
        wev = mainsb.tile([128, 4], F32, tag="wev")
        nc.vector.tensor_copy(out=wev, in_=wps0)
        w16 = mainsb.tile([128, 4], F16, tag="w16")
        nc.vector.tensor_tensor(out=w16, in0=wev, in1=wps8, op=ALU.add)
        w32 = mainsb.tile([128, 128], F16, tag="w32")
        w16b = bass.AP(tensor=w16.tensor, offset=w16.offset,
                       ap=[w16.ap[0], w16.ap[1], [0, 32]])
        nc.vector.tensor_copy(
            out=w32.rearrange("p (t r) -> p t r", r=32), in_=w16b)

        for u in range(8, NU):
            emit_l1(u)

        # ---------------- dots ----------------
        for g in range(4):
            dotp = prep.tile([128, 512], F32, tag="pre", name="dotp")
            for j in range(4):
                c = 4 * g + j
                for ht in range(4):
                    mm = nc.tensor.matmul(
                        dotp[32 * j:32 * (j + 1), :],
                        lhsT=w32[:, ht * 32:(ht + 1) * 32],
                        rhs=hid_tiles[2 * c + ht // 2][:, (ht % 2) * 512:
                                                       (ht % 2 + 1) * 512],
                        start=(ht == 0), stop=(ht == 3),
                        tile_position=(0, 32 * j))
                    if g == 0 and j == 0 and ht == 0:
                        add_dep_helper(mm.ins, l1_mms[NU - 1].ins, sync=False,
                                       reason="dots after L1 stream")
            oev = oevp.tile([128, 512], F32, tag="oev")
            nc.vector.tensor_copy(out=oev, in_=dotp)
            for jj in range(4):
                eng = nc.sync if jj % 2 == 0 else nc.scalar
                eng.dma_start(
                    out=out_d[(4 * g + jj) * 512:(4 * g + jj + 1) * 512]
                    .rearrange("(o b) -> o b", o=1),
                    in_=oev[32 * jj:32 * jj + 1, :])


def _get_nc():
    global _CACHED_NC
    if _CACHED_NC is None:
        _CACHED_NC = _build()
    return _CACHED_NC


def _prep_in_maps(inputs):
    f = lambda k: np.ascontiguousarray(np.asarray(inputs[k], np.float32))
    inputx = f("input")
    eq = float(np.asarray(inputs["eq_param"]).reshape(-1)[0])
    Wx1, bx1 = f("Wx1"), f("bx1")
    Wx2 = f("Wx2")

    wx1a = np.concatenate([Wx1, bx1[None, :]], axis=0)        # [4, 512]
    # wx1d[4i+k, j] = Wx1a[k, i*128+j]
    wx1d = np.ascontiguousarray(
        wx1a.reshape(4, 4, 128).transpose(1, 0, 2).reshape(16, 128)
    ).astype(np.float16)
    # wx2q[64*(b%2)+df, ((b//2)*4+t)*128+h] = Wx2[t*128+h, b*64+df]
    wx2q = np.ascontiguousarray(
        Wx2.reshape(4, 128, 8, 64)            # [t, h, b, df]
        .transpose(2, 3, 0, 1)                # [b, df, t, h]
        .reshape(4, 2, 64, 4, 128)            # [bp, s, df, t, h]
        .transpose(1, 2, 0, 3, 4)             # [s, df, bp, t, h]
        .reshape(128, 2048)
    ).astype(np.float16)

    wqad = np.empty((2, 384), np.float32)
    wq2 = np.empty((HQ, 3 * HQ), np.float32)
    bq2r = np.zeros((HQ, 3 * HQ), np.float32)
    qxad = np.ones((4, 12288), np.float32)
    qxc = np.empty((128, 192), np.float32)
    for br, (qk, w1k, b1k, w2k, b2k) in enumerate([
            ("quad_x0", "Wq01", "bq01", "Wq02", "bq02"),
            ("quad_x1", "Wq11", "bq11", "Wq12", "bq12"),
            ("quad_x2", "Wq21", "bq21", "Wq22", "bq22")]):
        wqad[0, br * 128:(br + 1) * 128] = f(w1k)[0]
        wqad[1, br * 128:(br + 1) * 128] = f(b1k)
        wq2[:, br * HQ:(br + 1) * HQ] = f(w2k)
        bq2r[0, br * HQ:(br + 1) * HQ] = f(b2k)
        sh = f(qk)[:, 0]                                  # all 8192 nodes
        for g in range(64):
            s = g % 2
            col = (br * 8 + g // 8) * 512 + ((g % 8) // 2) * 128
            qxad[2 * s, col:col + 128] = sh[g * 128:(g + 1) * 128]
        qxc[:, br * 64:(br + 1) * 64] = sh.reshape(64, 128).T
    wqad = wqad.astype(np.float16)
    eqb = np.full((128, 1), eq, np.float32)

    shared = dict(wx1d=wx1d, wx2q=wx2q, wqad=wqad,
                  wq2=wq2.astype(np.float16), bq2r=bq2r.astype(np.float16),
                  eqb=eqb, qxad=qxad.astype(np.float16),
                  qxc=np.ascontiguousarray(qxc))
    in_maps = []
    ones_row = np.ones((1, BL), np.float32)
    for c in range(NCORES):
        ish = inputx[c * BL:(c + 1) * BL]                        # [8192, 3]
        xTm = np.concatenate([ish.T, ones_row], axis=0)          # [4, 8192]
        m = dict(shared)
        m["xT"] = np.ascontiguousarray(xTm).astype(np.float16)
        in_maps.append(m)
    return in_maps


def _run(inputs, **kw):
    nc = _get_nc()
    in_maps = _prep_in_maps(inputs)
    res = run_bass_kernel_spmd(nc, in_maps, list(range(NCORES)), **kw)
    if _STAGE != "full":
        return res, res
    bx2 = np.asarray(inputs["bx2"], np.float64).reshape(-1)
    tail = np.asarray(res.results[0]["out"][BL:], np.float64).reshape(64, 8)
    rhs_vec = np.ascontiguousarray(tail.T).reshape(-1)        # [b*64+df]
    c_host = float(bx2 @ rhs_vec)
    out = np.concatenate([res.results[c]["out"][:BL].reshape(-1)
                          for c in range(NCORES)]).astype(np.float64)
    out = (out * OUT_SCALE + c_host).astype(np.float32)
    return out, res


def kernel(**inputs) -> np.ndarray:
    out, _ = _run(inputs)
    return out


def kernel_traced(**inputs):
    """Correctness + NTFF profile (exec_time_ns) in one run."""
    return _run(inputs, trace=True)


# revision 40
# speedup vs baseline: 1.0141x; 1.0141x over previous
"""Trainium2 Bass kernel for nn_DecGreenNet_product_CP3.

Reference computation:
    lhs  = tanh(input @ Wx1 + bx1) @ Wx2 + bx2          # [B, 512]
    s_i  = sum_n sin(pi*eq*qx_n) * mlp_i(qx_n)           # [8,16] per branch
    rhs  = einsum('bx,dx,fx->bdf', s_a, s_c, s_e)        # [512]
    out  = lhs @ rhs                                     # [B]

Algebraic restructuring (validated to ~3e-3 rel err):
    out[b] = tanh(input[b] @ Wx1 + bx1) @ (Wx2 @ rhs) + bx2 @ rhs
    s      = W2^T @ (h1tanh^T @ y) + (sum y) * b2   per quad branch
collapsing the dominant [B,512]x[512,512] GEMM into a matvec.

Implementation notes (v8):
  * Batch B sharded 8 ways (8192 rows/core). The quadrature branch is
    small (3 x 8192 x 128 tanh = 75% of one batch-shard's tanh work),
    so every core computes the FULL quadrature instead of exchanging
    partial sums: measured launch skew between the 8 cores is 40-60us
    per run, which made any cross-core reduction (AllReduce/AllGather/
    p2p remote_dma) the critical path. Fully local compute is ~20us
    slower in ScalarE terms but deterministic and variance-free.
  * The L1 GEMM has contraction K=4 (3 dims + bias) and runs as 4
    concurrent 32x128 row-tiles of the PE array (tile_position), with x
    and W replicated at partition offsets 0/32/64/96. The quadrature
    first layer (K=2) uses 2 row-tiles.
  * tanh runs in [128,1024] tiles from 3 rotating 2-bank PSUM slots;
    ScalarE is the roofline engine (56 tiles ~ 64us).
  * z = h1^T y runs with y as the stationary operand (1-column weight
    load) producing z as rows; a small SBUF gather DMA transposes the
    three z rows into matmul-rhs columns.
  * s lands directly in the [16, 24] einsum layout via M=16 matmuls on
    4 PE column-tiles. w = Wx2 @ rhs uses K=64 blocks on 2 row-tiles.
  * The final dot uses w replicated to 32 columns; each chunk lands as
    a 32-row PSUM stripe and 4 chunks fill a [128,512] bank evacuated
    with one DVE copy. Explicit scheduler deps pin the tensor-queue
    order quad -> L1 -> dots, with the s/einsum/w block slotted into
    the middle of the L1 stream (its inputs are ready long before).
  * The scalar c = bx2 @ rhs and the 2^36 fp16 descale are applied on
    the host; the kernel dumps the fp32 rhs vector in a 512-float tail
    of the output buffer.
"""

import os

import numpy as np

import concourse.bacc as bacc
import concourse.bass as bass
import concourse.mybir as mybir
import concourse.tile as tile
from concourse.bass_utils import run_bass_kernel_spmd
from concourse.tile import add_dep_helper

F32 = mybir.dt.float32
F16 = mybir.dt.float16
AF = mybir.ActivationFunctionType
ALU = mybir.AluOpType

NCORES = 8
B, DIN, H = 65536, 3, 512
N, HQ = 8192, 128
S0, RX = 8, 16
BL = B // NCORES          # 8192 batch rows per core
CH = 512                  # batch chunk (columns per L1 matmul)
NCH = BL // CH            # 16 chunks
NU = 2 * NCH              # 32 L1 units of [128, 1024]
NQU = 24                  # 24 quad units of [128, 1024] (3 br x 8 segs)

# fp16 scaling: w values are ~1e10-1e11; scale into fp16 range (exact pow2)
RC_SCALE = 2.0 ** -36     # applied to rhs_vec before the fp16 w-matmuls
OUT_SCALE = 2.0 ** 36     # undone on the host

# minimax odd polynomial for sin(t), t in [0, pi]: sin(t)=t*P(t^2), err<2e-5
SIN_C = (0.999984590176674, -0.16663258473611252, 8.312385898666645e-03,
         -1.9316230946716391e-04, 2.1732361127812407e-06)

_CACHED_NC = None
_STAGE = os.environ.get("K_STAGE", "full")  # sT | full


def _build():
    nc = bacc.Bacc("TRN2", target_bir_lowering=False, debug=False,
                   num_devices=NCORES)

    xT = nc.dram_tensor("xT", [DIN + 1, BL], F16, kind="ExternalInput").ap()
    wx1d = nc.dram_tensor("wx1d", [16, 128], F16, kind="ExternalInput").ap()
    wx2q = nc.dram_tensor("wx2q", [128, 2048], F16, kind="ExternalInput").ap()
    qxad = nc.dram_tensor("qxad", [4, 12288], F16, kind="ExternalInput").ap()
    wqad = nc.dram_tensor("wqad", [2, 384], F16, kind="ExternalInput").ap()
    qxc = nc.dram_tensor("qxc", [128, 192], F32, kind="ExternalInput").ap()
    wq2 = nc.dram_tensor("wq2", [HQ, 3 * HQ], F16, kind="ExternalInput").ap()
    bq2r = nc.dram_tensor("bq2r", [HQ, 3 * HQ], F16,
                          kind="ExternalInput").ap()
    eqb = nc.dram_tensor("eqb", [128, 1], F32, kind="ExternalInput").ap()
    out_d = nc.dram_tensor("out", [BL + 512], F32, kind="ExternalOutput").ap()

    global _APS
    _APS = (xT, wx1d, wx2q, qxad, wqad, qxc, wq2, bq2r, eqb, out_d)
    with tile.TileContext(nc) as tc:
        _body(nc, tc)
    nc.compile()
    return nc


def _body(nc, tc):
    xT, wx1d, wx2q, qxad, wqad, qxc, wq2, bq2r, eqb, out_d = _APS
    with (
        tc.tile_pool(name="const", bufs=1) as constp,
        tc.tile_pool(name="qsb", bufs=1) as qsb,
        tc.tile_pool(name="mainsb", bufs=1) as mainsb,
        tc.tile_pool(name="h1p", bufs=3) as h1p,
        tc.tile_pool(name="hidp", bufs=NU) as hidp,
        tc.tile_pool(name="oevp", bufs=2) as oevp,
        tc.tile_pool(name="prep", bufs=3, space="PSUM") as prep,
        tc.tile_pool(name="tinyp", bufs=1, space="PSUM") as tinyp,
        tc.tile_pool(name="outp", bufs=1, space="PSUM") as outp,
    ):
        ones128 = constp.tile([128, 1], F32)
        nc.vector.memset(ones128, 1.0)

        # ---------------- input DMAs ----------------
        # y-polynomial + xT on the scalar queue (idle until the first tanh)
        qxc_sb = qsb.tile([128, 192], F32, tag="qxc")
        nc.scalar.dma_start(out=qxc_sb, in_=qxc)
        eqb_sb = qsb.tile([128, 1], F32, tag="eqb")
        nc.scalar.dma_start(out=eqb_sb, in_=eqb)
        xTq = mainsb.tile([128, BL], F16, tag="xTq")
        for i in range(4):
            nc.scalar.dma_start(out=xTq[32 * i:32 * i + 4, :], in_=xT)

        # quad matmul inputs first on sync (they gate the first tanh);
        # qxaq arrives branch-by-branch so unit 0 isn't gated on all 96KB
        wqaq = qsb.tile([128, 384], F16, tag="wqaq")
        for s in range(2):
            nc.sync.dma_start(out=wqaq[32 * s:32 * s + 2, :], in_=wqad)
        qxaq = qsb.tile([128, 12288], F16, tag="qxaq")
        for br in range(3):
            for s in range(2):
                nc.sync.dma_start(
                    out=qxaq[32 * s:32 * s + 2, br * 4096:(br + 1) * 4096],
                    in_=qxad[2 * s:2 * s + 2, br * 4096:(br + 1) * 4096])
        wx1b = mainsb.tile([128, 128], F16, tag="wx1b")
        for i in range(4):
            nc.sync.dma_start(out=wx1b[32 * i:32 * i + 4, :],
                              in_=wx1d[4 * i:4 * i + 4, :])
        wq2_sb = qsb.tile([HQ, 3 * HQ], F16, tag="wq2")
        nc.sync.dma_start(out=wq2_sb, in_=wq2)
        bq2r_sb = qsb.tile([HQ, 3 * HQ], F16, tag="bq2r")
        nc.sync.dma_start(out=bq2r_sb, in_=bq2r)
        wx2q_sb = mainsb.tile([128, 2048], F16, tag="wx2q")
        nc.sync.dma_start(out=wx2q_sb, in_=wx2q)

        # ---------------- y = sin(pi*eq*qx) (DVE polynomial) ----------------
        eqpi = qsb.tile([128, 1], F32, tag="eqpi")
        nc.vector.tensor_scalar_mul(eqpi, eqb_sb, float(np.pi))
        tq = qsb.tile([128, 192], F32, tag="tq")
        nc.vector.tensor_scalar_mul(tq, qxc_sb, eqpi[:, 0:1])
        t2 = qsb.tile([128, 192], F32, tag="t2")
        nc.vector.tensor_tensor(out=t2, in0=tq, in1=tq, op=ALU.mult)
        pp = qsb.tile([128, 192], F32, tag="pp")
        c1, c3, c5, c7, c9 = [float(v) for v in SIN_C]
        nc.vector.tensor_scalar(out=pp, in0=t2, scalar1=c9, scalar2=c7,
                                op0=ALU.mult, op1=ALU.add)
        for cof in (c5, c3, c1):
            nc.vector.tensor_tensor(out=pp, in0=pp, in1=t2, op=ALU.mult)
            nc.vector.tensor_scalar_add(pp, pp, cof)
        y_sb = qsb.tile([128, 192], F16, tag="ysb")
        nc.vector.tensor_tensor(out=y_sb, in0=pp, in1=tq, op=ALU.mult)

        # ---------------- quad units ----------------
        # unit u = (br, seg): 8 node-tiles of 128 nodes; local tile k at
        # qpre cols (k%2)*512 + (k//2)*128, global node-tile g = seg*8+k.
        # zrow[0, br*128+h] accumulates z over all 64 node-tiles; cols
        # 384+br hold sum(y).
        # zsmall cols 0-23: per-unit z partials; cols 24-26: sum(y) (row 0)
        zsmall = tinyp.tile([128, 27], F32, tag="tiny")
        last_quad_mm = None
        # sum(y) scalars first: they are ready as soon as y is, and the
        # L1 stream dependency should not wait for them at the end
        for br in range(3):
            ysum = qsb.tile([128, 1], F32, tag=f"ysum{br}")
            nc.vector.tensor_reduce(
                out=ysum, in_=y_sb[:, br * 64:(br + 1) * 64],
                axis=mybir.AxisListType.X, op=ALU.add)
            nc.tensor.matmul(
                zsmall[0:1, 24 + br:25 + br], lhsT=ysum, rhs=ones128,
                start=True, stop=True, tile_position=(0, 0))
        for u in range(NQU):
            br, seg = u // 8, u % 8
            qpre = prep.tile([128, 1024], F32, tag="pre")
            for k in range(8):
                s, r = k % 2, k // 2
                col = (br * 8 + seg) * 512 + r * 128
                last_qpre_mm = nc.tensor.matmul(
                    qpre[:, s * 512 + r * 128:s * 512 + (r + 1) * 128],
                    lhsT=qxaq[32 * s:32 * s + 2, col:col + 128],
                    rhs=wqaq[32 * s:32 * s + 2, br * HQ:(br + 1) * HQ],
                    start=True, stop=True, tile_position=(32 * s, 0))
            h1 = h1p.tile([128, 1024], F16, tag="h1")
            nc.scalar.activation(out=h1, in_=qpre, func=AF.Tanh)
            # z partial for this unit: z[h] = sum_n h1[n, h] y[n]
            for k in range(8):
                s, r = k % 2, k // 2
                g = seg * 8 + k
                last_quad_mm = nc.tensor.matmul(
                    zsmall[:, u:u + 1],
                    lhsT=h1[:, s * 512 + r * 128:s * 512 + (r + 1) * 128],
                    rhs=y_sb[:, br * 64 + g:br * 64 + g + 1],
                    start=(k == 0), stop=(k == 7), tile_position=(0, 0))
        # fold the 8 segment partials per branch; sy scalars to row 0
        zq32 = qsb.tile([128, 6], F32, tag="zq32")
        nc.vector.memset(zq32, 0.0)
        for br in range(3):
            nc.vector.tensor_reduce(
                out=zq32[:, br:br + 1], in_=zsmall[:, br * 8:(br + 1) * 8],
                axis=mybir.AxisListType.X, op=ALU.add)
        nc.vector.tensor_copy(out=zq32[0:1, 3:6], in_=zsmall[0:1, 24:27])
        zcol = qsb.tile([128, 6], F16, tag="zcol")
        nc.vector.tensor_copy(out=zcol, in_=zq32)
        if _STAGE == "z":
            nc.sync.dma_start(
                out=out_d[0:768].rearrange("(p c) -> p c", c=6), in_=zq32)
            return

        # ---------------- main L1 units ----------------
        hid_tiles = {}
        l1_mms = {}

        def emit_l1(u):
            c, half = u // 2, u % 2
            pre = prep.tile([128, 1024], F32, tag="pre")
            for k in range(2):
                i = 2 * half + k          # PE row strip / h-tile index
                mm = nc.tensor.matmul(
                    pre[:, k * 512:(k + 1) * 512],
                    lhsT=wx1b[32 * i:32 * i + 4, :],
                    rhs=xTq[32 * i:32 * i + 4, c * CH:(c + 1) * CH],
                    start=True, stop=True, tile_position=(32 * i, 0))
                if u == 0 and k == 0:
                    # pin after the last quad PRE matmul, not the last z
                    # matmul: the final z group runs after the last quad
                    # tanh and would add ~1us to the main-stream handoff
                    add_dep_helper(mm.ins, last_qpre_mm.ins, sync=False,
                                   reason="L1 stream after quad matmuls")
                l1_mms[u] = mm
            hid = hidp.tile([128, 1024], F16, tag="hid")
            nc.scalar.activation(out=hid, in_=pre, func=AF.Tanh)
            hid_tiles[u] = hid

        for u in range(0, 8):
            emit_l1(u)

        # ---------------- s / einsum / w (all local) ----------------
        # slotted here in the tensor queue: its inputs are ready right
        # after the quad units, and the dots only need w much later.
        # s[b*16+x, br] lands in the [16, 24] einsum layout: column pair
        # cb = br*8+b -> PE col strip j=cb//6, col c=cb%6.
        sp16 = tinyp.tile([128, 6], F32, tag="tiny")
        first_s = None
        for cb in range(24):
            br, b = cb // 8, cb % 8
            j, c = cb // 6, cb % 6
            mm = nc.tensor.matmul(
                sp16[32 * j:32 * j + 16, c:c + 1],
                lhsT=wq2_sb[:, br * HQ + b * 16:br * HQ + (b + 1) * 16],
                rhs=zcol[:, br:br + 1], start=True, stop=False,
                tile_position=(0, 32 * j))
            if first_s is None:
                first_s = mm
                add_dep_helper(mm.ins, l1_mms[7].ins, sync=False,
                               reason="s block mid-L1-stream")
            nc.tensor.matmul(
                sp16[32 * j:32 * j + 16, c:c + 1],
                lhsT=bq2r_sb[:, br * HQ + b * 16:br * HQ + (b + 1) * 16],
                rhs=zcol[:, 3 + br:4 + br], start=False, stop=True,
                tile_position=(0, 32 * j))
        sSB = qsb.tile([128, 6], F32, tag="sSB")
        nc.vector.tensor_copy(out=sSB, in_=sp16)
        sT = mainsb.tile([16, 24], F32, tag="sT")
        for j in range(4):
            nc.sync.dma_start(out=sT[0:16, 6 * j:6 * (j + 1)],
                              in_=sSB[32 * j:32 * j + 16, :])
        if _STAGE == "sT":
            nc.sync.dma_start(
                out=out_d[0:384].rearrange("(p c) -> p c", c=24), in_=sT)
            return
        # E[x, d*8+f] = s_c[d,x] * s_e[f,x]
        sc_ap = sT[:, 8:16]
        se_ap = sT[:, 16:24]
        in0 = bass.AP(tensor=sc_ap.tensor, offset=sc_ap.offset,
                      ap=[sc_ap.ap[0], sc_ap.ap[1], [0, 8]])
        in1 = bass.AP(tensor=se_ap.tensor, offset=se_ap.offset,
                      ap=[se_ap.ap[0], [0, 8], se_ap.ap[1]])
        E_sb = mainsb.tile([16, 64], F32, tag="E")
        nc.vector.tensor_tensor(
            out=E_sb.rearrange("p (d f) -> p d f", f=8),
            in0=in0, in1=in1, op=ALU.mult)
        # rhsp[d*8+f, b] = sum_x E[x, df] * s_a[b, x]
        rhsp = tinyp.tile([64, 8], F32, tag="tiny")
        nc.tensor.matmul(rhsp, lhsT=E_sb, rhs=sT[:, 0:8],
                         start=True, stop=True, tile_position=(0, 0))
        r16 = mainsb.tile([64, 8], F16, tag="r16")
        nc.vector.tensor_scalar_mul(r16, rhsp, float(RC_SCALE))
        # fp32 rhs vector to the output tail (host computes bx2 @ rhs)
        rf32 = mainsb.tile([64, 8], F32, tag="rf32")
        nc.vector.tensor_copy(out=rf32, in_=rhsp)
        nc.sync.dma_start(
            out=out_d[BL:BL + 512].rearrange("(p c) -> p c", c=8),
            in_=rf32)
        # replicate r16 to partitions 64-127 for the 2-strip w matmuls:
        # strip s (partitions 64s+df) holds cols q -> b = 2q+s
        rcol2 = mainsb.tile([128, 4], F16, tag="rcol2")
        r16v = r16.rearrange("p (q b2) -> p b2 q", b2=2)
        nc.sync.dma_start(out=rcol2[0:64, :], in_=r16v[:, 0:1, :])
        nc.sync.dma_start(out=rcol2[64:128, :], in_=r16v[:, 1:2, :])
        # w[t*128+h] = sum_b sum_df Wx2[t*128+h, b*64+df] rhs[b*64+df]
        wps0 = tinyp.tile([128, 4], F32, tag="tiny")
        wps8 = outp.tile([128, 4], F32, tag="wps8")
        wtile = [wps0, wps8]
        for t in range(4):
            for bp in range(4):
                for s in range(2):
                    nc.tensor.matmul(
                        wtile[s][:, t:t + 1],
                        lhsT=wx2q_sb[64 * s:64 * s + 64,
                                     (bp * 4 + t) * 128:(bp * 4 + t + 1) * 128],
                        rhs=rcol2[64 * s:64 * s + 64, bp:bp + 1],
                        start=(bp == 0), stop=(bp == 3),
                        tile_position=(64 * s, 0))
        # w16 = wps0 + wps8, then replicate each column 32x for the dots
        wev = mainsb.tile([128, 4], F32, tag="wev")
        nc.vector.tensor_copy(out=wev, in_=wps0)
        w16 = mainsb.tile([128, 4], F16, tag="w16")
        nc.vector.tensor_tensor(out=w16, in0=wev, in1=wps8, op=ALU.add)
        w32 = mainsb.tile([128, 128], F16, tag="w32")
        w16b = bass.AP(tensor=w16.tensor, offset=w16.offset,
                       ap=[w16.ap[0], w16.ap[1], [0, 32]])
        nc.vector.tensor_copy(
            out=w32.rearrange("p (t r) -> p t r", r=32), in_=w16b)

        for u in range(8, NU):
            emit_l1(u)

        # ---------------- dots ----------------
        for g in range(4):
            dotp = prep.tile([128, 512], F32, tag="pre", name="dotp")
            for j in range(4):
                c = 4 * g + j
                for ht in range(4):
                    mm = nc.tensor.matmul(
                        dotp[32 * j:32 * (j + 1), :],
                        lhsT=w32[:, ht * 32:(ht + 1) * 32],
                        rhs=hid_tiles[2 * c + ht // 2][:, (ht % 2) * 512:
                                                       (ht % 2 + 1) * 512],
                        start=(ht == 0), stop=(ht == 3),
                        tile_position=(0, 32 * j))
                    if g == 0 and j == 0 and ht == 0:
                        add_dep_helper(mm.ins, l1_mms[NU - 1].ins, sync=False,
                                       reason="dots after L1 stream")
            oev = oevp.tile([128, 512], F32, tag="oev")
            nc.vector.tensor_copy(out=oev, in_=dotp)
            for jj in range(4):
                eng = nc.sync if jj % 2 == 0 else nc.scalar
                eng.dma_start(
                    out=out_d[(4 * g + jj) * 512:(4 * g + jj + 1) * 512]
                    .rearrange("(o b) -> o b", o=1),
                    in_=oev[32 * jj:32 * jj + 1, :])


def _get_nc():
    global _CACHED_NC
    if _CACHED_NC is None:
        _CACHED_NC = _build()
    return _CACHED_NC


def _prep_in_maps(inputs):
    f = lambda k: np.ascontiguousarray(np.asarray(inputs[k], np.float32))
    inputx = f("input")
    eq = float(np.asarray(inputs["eq_param"]).reshape(-1)[0])
    Wx1, bx1 = f("Wx1"), f("bx1")
    Wx2 = f("Wx2")

    wx1a = np.concatenate([Wx1, bx1[None, :]], axis=0)        # [4, 512]
    # wx1d[4i+k, j] = Wx1a[k, i*128+j]
    wx1d = np.ascontiguousarray(
        wx1a.reshape(4, 4, 128).transpose(1, 0, 2).reshape(16, 128)
    ).astype(np.float16)
    # wx2q[64*(b%2)+df, ((b//2)*4+t)*128+h] = Wx2[t*128+h, b*64+df]
    wx2q = np.ascontiguousarray(
        Wx2.reshape(4, 128, 8, 64)            # [t, h, b, df]
        .transpose(2, 3, 0, 1)                # [b, df, t, h]
        .reshape(4, 2, 64, 4, 128)            # [bp, s, df, t, h]
        .transpose(1, 2, 0, 3, 4)             # [s, df, bp, t, h]
        .reshape(128, 2048)
    ).astype(np.float16)

    wqad = np.empty((2, 384), np.float32)
    wq2 = np.empty((HQ, 3 * HQ), np.float32)
    bq2r = np.zeros((HQ, 3 * HQ), np.float32)
    qxad = np.ones((4, 12288), np.float32)
    qxc = np.empty((128, 192), np.float32)
    for br, (qk, w1k, b1k, w2k, b2k) in enumerate([
            ("quad_x0", "Wq01", "bq01", "Wq02", "bq02"),
            ("quad_x1", "Wq11", "bq11", "Wq12", "bq12"),
            ("quad_x2", "Wq21", "bq21", "Wq22", "bq22")]):
        wqad[0, br * 128:(br + 1) * 128] = f(w1k)[0]
        wqad[1, br * 128:(br + 1) * 128] = f(b1k)
        wq2[:, br * HQ:(br + 1) * HQ] = f(w2k)
        bq2r[0, br * HQ:(br + 1) * HQ] = f(b2k)
        sh = f(qk)[:, 0]                                  # all 8192 nodes
        for g in range(64):
            s = g % 2
            col = (br * 8 + g // 8) * 512 + ((g % 8) // 2) * 128
            qxad[2 * s, col:col + 128] = sh[g * 128:(g + 1) * 128]
        qxc[:, br * 64:(br + 1) * 64] = sh.reshape(64, 128).T
    wqad = wqad.astype(np.float16)
    eqb = np.full((128, 1), eq, np.float32)

    shared = dict(wx1d=wx1d, wx2q=wx2q, wqad=wqad,
                  wq2=wq2.astype(np.float16), bq2r=bq2r.astype(np.float16),
                  eqb=eqb, qxad=qxad.astype(np.float16),
                  qxc=np.ascontiguousarray(qxc))
    in_maps = []
    ones_row = np.ones((1, BL), np.float32)
    for c in range(NCORES):
        ish = inputx[c * BL:(c + 1) * BL]                        # [8192, 3]
        xTm = np.concatenate([ish.T, ones_row], axis=0)          # [4, 8192]
        m = dict(shared)
        m["xT"] = np.ascontiguousarray(xTm).astype(np.float16)
        in_maps.append(m)
    return in_maps


def _run(inputs, **kw):
    nc = _get_nc()
    in_maps = _prep_in_maps(inputs)
    res = run_bass_kernel_spmd(nc, in_maps, list(range(NCORES)), **kw)
    if _STAGE != "full":
        return res, res
    bx2 = np.asarray(inputs["bx2"], np.float64).reshape(-1)
    tail = np.asarray(res.results[0]["out"][BL:], np.float64).reshape(64, 8)
    rhs_vec = np.ascontiguousarray(tail.T).reshape(-1)        # [b*64+df]
    c_host = float(bx2 @ rhs_vec)
    out = np.concatenate([res.results[c]["out"][:BL].reshape(-1)
                          for c in range(NCORES)]).astype(np.float64)
    out = (out * OUT_SCALE + c_host).astype(np.float32)
    return out, res


def kernel(**inputs) -> np.ndarray:
    out, _ = _run(inputs)
    return out


def kernel_traced(**inputs):
    """Correctness + NTFF profile (exec_time_ns) in one run."""
    return _run(inputs, trace=True)
